# revision 18
# baseline (speedup 1.0000x reference)
"""DocRed model Trainium2 kernel.

Sharding: 8 cores = 4 docs x 2 pair-halves (276 pairs each). Each core runs
the full per-doc pipeline for its pairs: segment pooling (one-hot matmuls),
attention gathers, head-pair attention, rel einsum, head/tail projections,
bilinear GEMM against the full [49152,768] weight, and classifier logits.
All matmuls run in float32r (fp32 rounded to 12-bit mantissa, 1 cyc/row on
the PE at N>=256). Constant operands are pre-rounded host-side and staged as
float32r-typed DRAM tensors; on-device producers write f32r directly.
"""
import sys
sys.path.insert(0, '/opt/trn_rl_repo')
import numpy as np

import concourse.bass as bass
import concourse.tile as tile
from concourse import bacc, mybir
from concourse.bass_utils import run_bass_kernel_spmd

# Problem dims (hardcoded per contract)
B, M, E, R = 4, 72, 24, 552
L, H, A = 1024, 1024, 16
EMB, BLK, OUT, NC_ = 768, 64, 768, 97
KBL = EMB // BLK               # 12 blocks
NPC = R // 2                   # 276 pairs per core
KIJ = EMB * BLK                # 49152 contraction rows
NCHUNK = KIJ // 128            # 384 chunks
F32R = mybir.dt.float32r
F32 = mybir.dt.float32
Tanh = mybir.ActivationFunctionType.Tanh
Exp = mybir.ActivationFunctionType.Exp
Ln = mybir.ActivationFunctionType.Ln
Copy = mybir.ActivationFunctionType.Copy


def f32r_round(a):
    u = np.ascontiguousarray(a, np.float32).view(np.uint32)
    u = ((u.astype(np.uint64) + 0x400) & 0xFFFFF800).astype(np.uint32)
    return np.ascontiguousarray(u.view(np.float32))


def build_program():
    nc = bacc.Bacc("TRN2", target_bir_lowering=False, debug=False, num_devices=1)
    d = {}
    def di(name, shape, dt=F32R):
        d[name] = nc.dram_tensor(name, shape, dt, kind="ExternalInput")
        return d[name]
    # weights / constants (same data on all cores)
    di("wbil", [KIJ, OUT])
    di("whead", [2 * H, EMB]); di("wtail", [2 * H, EMB])
    di("wcls", [OUT, NC_]); di("wbin", [OUT, 1])
    di("ident", [128, 128])
    di("sel16", [32, 16, 128]); di("sel64", [64, 128])
    di("onesl", [128, 1])            # f32r ones for row-sum lhsT
    di("repl", [1, 128], F32)        # f32 ones row for inv_s replication
    di("bh", [128, 6], F32); di("bt", [128, 6], F32); di("bbil", [128, 6], F32)
    di("bcls", [NC_, 1], F32); di("bbin", [1, 1], F32)
    # per-doc
    di("seqd", [L, H])               # lhsT for rel (l on partitions)
    di("attnT", [M, A * L])          # [m, a*L+l]
    di("entl", [M, H], F32)
    di("smt", [M, E])                # segment one-hot lhsT
    di("eadd", [E, 1], F32)          # +1 for empty segments (Ln bias)
    # per-core (pair-half)
    di("ghp", [E, NPC]); di("gtp", [E, NPC])   # folded one-hots
    di("hh", [E, NPC]); di("htt", [E, NPC])    # plain one-hots for ent gathers
    # outputs
    embd = nc.dram_tensor("embT", [OUT, NPC], F32, kind="ExternalOutput")
    clsd = nc.dram_tensor("clsT", [NC_, NPC], F32, kind="ExternalOutput")
    bind = nc.dram_tensor("binT", [1, NPC], F32, kind="ExternalOutput")

    with tile.TileContext(nc) as tc:
        with tc.tile_pool(name="const", bufs=1) as cpool, \
             tc.tile_pool(name="big", bufs=1) as bpool, \
             tc.tile_pool(name="attns", bufs=2) as atpool, \
             tc.tile_pool(name="seqs", bufs=1) as sqpool, \
             tc.tile_pool(name="wstr", bufs=3) as spool, \
             tc.tile_pool(name="wstr2", bufs=4) as s2pool, \
             tc.tile_pool(name="prod", bufs=3) as prpool, \
             tc.tile_pool(name="sums", bufs=4) as supool, \
             tc.tile_pool(name="hrep", bufs=3) as hrpool, \
             tc.tile_pool(name="blc", bufs=3) as blpool, \
             tc.tile_pool(name="acc", bufs=6, space="PSUM") as pacc, \
             tc.tile_pool(name="work", bufs=2, space="PSUM") as pwork:

            # ---- load constants ----
            def load(name, shape, dt=F32R, src=None):
                t = cpool.tile(shape, dt, tag=name)
                nc.sync.dma_start(t[:], src if src is not None else d[name].ap())
                return t
            ident = load("ident", [128, 128])
            sel16 = load("sel16", [32, 16, 128])
            sel64 = load("sel64", [64, 128])
            onesl = load("onesl", [128, 1])
            repl = load("repl", [1, 128], F32)
            smt = load("smt", [M, E])
            ghp = load("ghp", [E, NPC]); gtp = load("gtp", [E, NPC])
            hh = load("hh", [E, NPC]); htt = load("htt", [E, NPC])
            bh = load("bh", [128, 6], F32); bt = load("bt", [128, 6], F32)
            bbil = load("bbil", [128, 6], F32)
            bcls = load("bcls", [NC_, 1], F32); bbin = load("bbin", [1, 1], F32)
            eadd = load("eadd", [E, 1], F32)
            entl = load("entl", [M, H], F32)
            wcls = load("wcls", [128, 6, NC_],
                        src=d["wcls"].ap().rearrange("(c p) n -> p c n", p=128))
            wbin = load("wbin", [128, 6, 1],
                        src=d["wbin"].ap().rearrange("(c p) n -> p c n", p=128))

            # ---- pooling: ent_emb = ln(sum exp) ; asum = S @ attn ----
            exp_x = bpool.tile([M, H], F32R, tag="expx")
            nc.scalar.activation(exp_x[:], entl[:], Exp)
            lse = bpool.tile([E, H], F32R, tag="lse")
            for j in range(2):
                ps = pwork.tile([E, 512], F32, tag="w")
                nc.tensor.matmul(ps[:], smt[:], exp_x[:, 512 * j:512 * (j + 1)],
                                 start=True, stop=True)
                nc.scalar.activation(lse[:, 512 * j:512 * (j + 1)], ps[:], Ln,
                                     bias=eadd[:])

            # ---- ha/ta gathers + ht_un, streamed per attention head a ----
            htf = bpool.tile([128, 8, NPC], F32, tag="htf")      # f32 accumulator
            htun = bpool.tile([128, 8, NPC], F32R, tag="htun")   # final f32r
            for a in range(A):
                at = atpool.tile([M, 1024], F32R, tag="attn", name=f"at{a}")
                nc.sync.dma_start(at[:], d["attnT"].ap()[:, 1024 * a:1024 * (a + 1)])
                asb = prpool.tile([E, 1024], F32R, tag="asb", name=f"as{a}")
                for jj in range(2):
                    ps = pwork.tile([E, 512], F32, tag="w", name=f"as{a}_{jj}")
                    nc.tensor.matmul(ps[:], smt[:], at[:, 512 * jj:512 * (jj + 1)],
                                     start=True, stop=True)
                    nc.scalar.activation(asb[:, 512 * jj:512 * (jj + 1)], ps[:],
                                         Copy)
                for r in range(8):       # l-range
                    sl = asb[:, 128 * r:128 * r + 128]
                    pha = pacc.tile([128, NPC], F32, tag="acc", name=f"ha{a}_{r}")
                    nc.tensor.matmul(pha[:], sl, ghp[:], start=True, stop=True)
                    pta = pacc.tile([128, NPC], F32, tag="acc", name=f"ta{a}_{r}")
                    nc.tensor.matmul(pta[:], sl, gtp[:], start=True, stop=True)
                    hsb = prpool.tile([128, NPC], F32, tag="hsb", name=f"hs{a}_{r}")
                    nc.scalar.activation(hsb[:], pha[:], Copy)
                    if a == 0:
                        nc.vector.tensor_mul(htf[:, r, :], hsb[:], pta[:])
                    else:
                        p = prpool.tile([128, NPC], F32, tag="p", name=f"p{a}_{r}")
                        nc.vector.tensor_mul(p[:], hsb[:], pta[:])
                        dst = htun if a == A - 1 else htf
                        eng = nc.gpsimd if (a % 2 == 0 and a != A - 1) else nc.vector
                        eng.tensor_add(dst[:, r, :], htf[:, r, :], p[:])

            # ---- normalization scale: inv_s replicated over partitions ----
            sps = pwork.tile([1, NPC], F32, tag="w")
            for j in range(8):
                nc.tensor.matmul(sps[:], onesl[:], htun[:, j, :],
                                 start=(j == 0), stop=(j == 7))
            seps = supool.tile([1, NPC], F32, tag="s")
            nc.vector.tensor_scalar_add(seps[:], sps[:], 1e-5)
            invs = supool.tile([1, NPC], F32, tag="s")
            nc.vector.reciprocal(invs[:], seps[:])
            irep_ps = pwork.tile([128, NPC], F32, tag="w")
            nc.tensor.matmul(irep_ps[:], repl[:], invs[:], start=True, stop=True)
            irep = bpool.tile([128, NPC], F32, tag="irepsb")
            nc.scalar.activation(irep[:], irep_ps[:], Copy)

            # ---- rel (transposed) + scale ----
            relT = bpool.tile([128, 8, NPC], F32R, tag="relT")
            for og in range(4):
                sq = sqpool.tile([128, 8, 256], F32R, tag="seq")
                nc.sync.dma_start(
                    sq[:], d["seqd"].ap().rearrange("(c p) h -> p c h", p=128)
                    [:, :, 256 * og:256 * (og + 1)])
                for oo in range(2):
                    o = 2 * og + oo
                    ps = pwork.tile([128, NPC], F32, tag="w", name=f"rl{og}{oo}")
                    for j in range(8):
                        nc.tensor.matmul(ps[:], sq[:, j, 128 * oo:128 * (oo + 1)],
                                         htun[:, j, :], start=(j == 0), stop=(j == 7))
                    nc.vector.tensor_mul(relT[:, o, :], ps[:], irep[:])

            # ---- projections: hsT/tsT = tanh(W^T [ent;rel] + b) ----
            hsT = bpool.tile([128, 6, NPC], F32R, tag="hsT")
            tsT = bpool.tile([128, 6, NPC], F32R, tag="tsT")
            for wname, hot, dst, bias in (("whead", hh, hsT, bh),
                                          ("wtail", htt, tsT, bt)):
                accs = [pacc.tile([128, NPC], F32, tag="acc", name=f"pj{o}")
                        for o in range(6)]
                for j in range(16):
                    wc = spool.tile([128, EMB], F32R, tag="wstr")
                    nc.sync.dma_start(wc[:], d[wname].ap()[128 * j:128 * (j + 1), :])
                    if j < 8:
                        eps_ = pwork.tile([128, NPC], F32, tag="w",
                                          name=f"eg{wname}{j}")
                        nc.tensor.matmul(eps_[:], lse[:, 128 * j:128 * (j + 1)],
                                         hot[:], start=True, stop=True)
                        ent_sb = prpool.tile([128, NPC], F32R, tag="entg",
                                             name=f"eg2{wname}{j}")
                        nc.scalar.activation(ent_sb[:], eps_[:], Copy)
                        rhs = ent_sb[:]
                    else:
                        rhs = relT[:, j - 8, :]
                    for o in range(6):
                        nc.tensor.matmul(accs[o][:], wc[:, 128 * o:128 * (o + 1)],
                                         rhs, start=(j == 0), stop=(j == 15))
                for o in range(6):
                    nc.scalar.activation(dst[:, o, :], accs[o][:], Tanh,
                                         bias=bias[:, o:o + 1])

            # ---- bilinear GEMM: embT[o, n] = sum_c W[c-chunk]^T bl^T[c-chunk] ----
            eaccs = [pacc.tile([128, NPC], F32, tag="acc", name=f"ea{o}")
                     for o in range(6)]
            ts2c = None
            h2sb32 = None
            for c in range(NCHUNK):
                k = c // 32
                if c % 32 == 0:      # replicate ts k-block on both halves
                    t2a = pwork.tile([64, NPC], F32, tag="w", name=f"t2a{k}")
                    nc.tensor.matmul(t2a[:],
                                     ident[:, 64 * (k % 2):64 * (k % 2) + 64],
                                     tsT[:, k // 2, :], start=True, stop=True)
                    t2sb = blpool.tile([64, NPC], F32R, tag="t2sb", name=f"t2s{k}")
                    nc.vector.tensor_copy(t2sb[:], t2a[:])
                    tps = pwork.tile([128, NPC], F32, tag="w", name=f"tp{k}")
                    nc.tensor.matmul(tps[:], sel64[:], t2sb[:],
                                     start=True, stop=True)
                    ts2c = hrpool.tile([128, NPC], F32R, tag="ts2c", name=f"t2{k}")
                    nc.scalar.activation(ts2c[:], tps[:], Copy)
                wt = spool.tile([128, OUT], F32R, tag="wstr", name=f"wb{c}")
                nc.sync.dma_start(wt[:], d["wbil"].ap()[128 * c:128 * (c + 1), :])
                if c % 16 == 0:
                    q16 = c // 16
                    hp2 = pwork.tile([32, NPC], F32, tag="w", name=f"hp2_{q16}")
                    nc.tensor.matmul(hp2[:],
                                     ident[:, 32 * (q16 % 4):32 * (q16 % 4) + 32],
                                     hsT[:, q16 // 4, :], start=True, stop=True)
                    h2sb32 = blpool.tile([32, NPC], F32R, tag="h2sb",
                                         name=f"h2s{q16}")
                    nc.vector.tensor_copy(h2sb32[:], hp2[:])
                hp = pwork.tile([128, NPC], F32, tag="w", name=f"hp{c}")
                nc.tensor.matmul(hp[:], sel16[:, c % 16, :], h2sb32[:],
                                 start=True, stop=True)
                hr = hrpool.tile([128, NPC], F32R, tag="hr")
                nc.scalar.activation(hr[:], hp[:], Copy)
                bl = blpool.tile([128, NPC], F32R, tag="bl")
                nc.vector.tensor_mul(bl[:], ts2c[:], hr[:])
                for o in range(6):
                    nc.tensor.matmul(eaccs[o][:], wt[:, 128 * o:128 * (o + 1)],
                                     bl[:], start=(c == 0), stop=(c == NCHUNK - 1))

            # ---- outputs ----
            embT = bpool.tile([128, 6, NPC], F32R, tag="embT")
            for o in range(6):
                nc.vector.tensor_scalar_add(embT[:, o, :], eaccs[o][:],
                                            bbil[:, o:o + 1])
            nc.sync.dma_start(
                embd.ap().rearrange("(c p) n -> p c n", p=128),
                embT[:].bitcast(F32))
            pcls = pwork.tile([NC_, NPC], F32, tag="w")
            for j in range(6):
                nc.tensor.matmul(pcls[:], wcls[:, j, :], embT[:, j, :],
                                 start=(j == 0), stop=(j == 5))
            clsT = bpool.tile([NC_, NPC], F32, tag="clsT")
            nc.vector.tensor_scalar_add(clsT[:], pcls[:], bcls[:])
            nc.sync.dma_start(clsd.ap(), clsT[:])
            pbin = pwork.tile([1, NPC], F32, tag="w")
            for j in range(6):
                nc.tensor.matmul(pbin[:], wbin[:, j, :], embT[:, j, :],
                                 start=(j == 0), stop=(j == 5))
            binT = bpool.tile([1, NPC], F32, tag="binT")
            nc.vector.tensor_scalar_add(binT[:], pbin[:], bbin[:])
            nc.sync.dma_start(bind.ap(), binT[:])

    nc.compile()
    return nc


_NC_CACHE = None


def _get_program():
    global _NC_CACHE
    if _NC_CACHE is None:
        _NC_CACHE = build_program()
    return _NC_CACHE


def _one_hot_T(idx, n):
    """[n, len(idx)] one-hot: out[e, r] = (idx[r] == e)."""
    out = np.zeros((n, len(idx)), np.float32)
    out[idx, np.arange(len(idx))] = 1.0
    return out


def kernel(seq_lhs, ent_lhs, ent_to_seq_attn, entity_id_labels, hts,
           W_head, b_head, W_tail, b_tail, W_bil, b_bil,
           W_cls, b_cls, W_bin, b_bin):
    seq_lhs = np.asarray(seq_lhs, np.float32)
    ent_lhs = np.asarray(ent_lhs, np.float32)
    ent_to_seq_attn = np.asarray(ent_to_seq_attn, np.float32)
    entity_id_labels = np.asarray(entity_id_labels)
    hts = np.asarray(hts)

    nc = _get_program()

    shared = {
        "wbil": f32r_round(W_bil),
        "whead": f32r_round(W_head), "wtail": f32r_round(W_tail),
        "wcls": f32r_round(W_cls), "wbin": f32r_round(W_bin),
        "ident": np.eye(128, dtype=np.float32),
        "sel16": (np.arange(32)[:, None, None]
                  == 2 * np.arange(16)[None, :, None]
                  + (np.arange(128)[None, None, :] >= 64))
                 .astype(np.float32),
        "sel64": (np.arange(64)[:, None] == np.arange(128)[None, :] % 64)
                .astype(np.float32),
        "onesl": np.ones((128, 1), np.float32),
        "repl": np.ones((1, 128), np.float32),
        "bh": np.asarray(b_head, np.float32).reshape(6, 128).T.copy(),
        "bt": np.asarray(b_tail, np.float32).reshape(6, 128).T.copy(),
        "bbil": np.asarray(b_bil, np.float32).reshape(6, 128).T.copy(),
        "bcls": np.asarray(b_cls, np.float32).reshape(NC_, 1),
        "bbin": np.asarray(b_bin, np.float32).reshape(1, 1),
    }
    in_maps = []
    for c in range(8):
        dcc, h = divmod(c, 2)
        labels = entity_id_labels[dcc]
        cnt = np.bincount(labels, minlength=E).astype(np.float32)
        inv_cnt = 1.0 / np.maximum(cnt, 1.0)
        p0 = NPC * h
        hd = hts[dcc, p0:p0 + NPC, 0]
        td = hts[dcc, p0:p0 + NPC, 1]
        m = dict(shared)
        m["seqd"] = f32r_round(seq_lhs[dcc])
        m["attnT"] = f32r_round(
            ent_to_seq_attn[dcc].transpose(1, 0, 2).reshape(M, A * L))
        m["entl"] = ent_lhs[dcc]
        m["smt"] = _one_hot_T(labels, E).T.copy()     # [M, E]
        m["eadd"] = (cnt == 0).astype(np.float32).reshape(E, 1)
        m["ghp"] = f32r_round(_one_hot_T(hd, E) * (inv_cnt / A)[:, None])
        m["gtp"] = f32r_round(_one_hot_T(td, E) * inv_cnt[:, None])
        m["hh"] = _one_hot_T(hd, E)
        m["htt"] = _one_hot_T(td, E)
        in_maps.append(m)

    res = run_bass_kernel_spmd(nc, in_maps, core_ids=list(range(8)))

    embeds = np.empty((B * R, OUT), np.float32)
    cls_l = np.empty((B * R, NC_), np.float32)
    bin_l = np.empty((B * R, 1), np.float32)
    for c in range(8):
        dcc, h = divmod(c, 2)
        r0 = R * dcc + NPC * h
        embeds[r0:r0 + NPC] = res.results[c]["embT"].T
        cls_l[r0:r0 + NPC] = res.results[c]["clsT"].T
        bin_l[r0:r0 + NPC] = res.results[c]["binT"].T
    return embeds, cls_l, bin_l


# revision 19
# speedup vs baseline: 1.1994x; 1.1994x over previous
"""DocRed model Trainium2 kernel.

Sharding: 8 cores = 4 docs x 2 pair-halves (276 pairs each). Each core runs
the full per-doc pipeline for its pairs: segment pooling (one-hot matmuls),
attention gathers, head-pair attention, rel einsum, head/tail projections,
bilinear GEMM against the full [49152,768] weight, and classifier logits.
All matmuls run in float32r (fp32 rounded to 12-bit mantissa, 1 cyc/row on
the PE at N>=256). Constant operands are pre-rounded host-side and staged as
float32r-typed DRAM tensors; on-device producers write f32r directly.
"""
import sys
sys.path.insert(0, '/opt/trn_rl_repo')
import numpy as np

import concourse.bass as bass
import concourse.tile as tile
from concourse import bacc, mybir
from concourse.bass_utils import run_bass_kernel_spmd

# Problem dims (hardcoded per contract)
B, M, E, R = 4, 72, 24, 552
L, H, A = 1024, 1024, 16
EMB, BLK, OUT, NC_ = 768, 64, 768, 97
KBL = EMB // BLK               # 12 blocks
NPC = R // 2                   # 276 pairs per core
KIJ = EMB * BLK                # 49152 contraction rows
NCHUNK = KIJ // 128            # 384 chunks
F32R = mybir.dt.float32r
F32 = mybir.dt.float32
Tanh = mybir.ActivationFunctionType.Tanh
Exp = mybir.ActivationFunctionType.Exp
Ln = mybir.ActivationFunctionType.Ln
Copy = mybir.ActivationFunctionType.Copy


def f32r_round(a):
    u = np.ascontiguousarray(a, np.float32).view(np.uint32)
    u = ((u.astype(np.uint64) + 0x400) & 0xFFFFF800).astype(np.uint32)
    return np.ascontiguousarray(u.view(np.float32))


def build_program():
    nc = bacc.Bacc("TRN2", target_bir_lowering=False, debug=False, num_devices=1)
    d = {}
    def di(name, shape, dt=F32R):
        d[name] = nc.dram_tensor(name, shape, dt, kind="ExternalInput")
        return d[name]
    # weights / constants (same data on all cores)
    di("wbil", [KIJ, OUT])
    di("whead", [2 * H, EMB]); di("wtail", [2 * H, EMB])
    di("wcls", [OUT, NC_]); di("wbin", [OUT, 1])
    di("ident", [128, 128])
    di("sel16", [32, 16, 128]); di("sel64", [64, 128])
    di("onesl", [128, 1])            # f32r ones for row-sum lhsT
    di("repl", [1, 128], F32)        # f32 ones row for inv_s replication
    di("bh", [128, 6], F32); di("bt", [128, 6], F32); di("bbil", [128, 6], F32)
    di("bcls", [NC_, 1], F32); di("bbin", [1, 1], F32)
    # per-doc
    di("seqd", [L, H])               # lhsT for rel (l on partitions)
    di("attnT", [M, A * L])          # [m, a*L+l]
    di("entl", [M, H], F32)
    di("smt", [M, E])                # segment one-hot lhsT
    di("eadd", [E, 1], F32)          # +1 for empty segments (Ln bias)
    # per-core (pair-half)
    di("ghp", [E, NPC]); di("gtp", [E, NPC])   # folded one-hots
    di("hh", [E, NPC]); di("htt", [E, NPC])    # plain one-hots for ent gathers
    # outputs
    embd = nc.dram_tensor("embT", [OUT, NPC], F32, kind="ExternalOutput")
    clsd = nc.dram_tensor("clsT", [NC_, NPC], F32, kind="ExternalOutput")
    bind = nc.dram_tensor("binT", [1, NPC], F32, kind="ExternalOutput")

    with tile.TileContext(nc) as tc:
        with tc.tile_pool(name="const", bufs=1) as cpool, \
             tc.tile_pool(name="big", bufs=1) as bpool, \
             tc.tile_pool(name="attns", bufs=2) as atpool, \
             tc.tile_pool(name="seqs", bufs=1) as sqpool, \
             tc.tile_pool(name="wstr", bufs=12) as spool, \
             tc.tile_pool(name="prod", bufs=3) as prpool, \
             tc.tile_pool(name="sums", bufs=2) as supool, \
             tc.tile_pool(name="hrep", bufs=3) as hrpool, \
             tc.tile_pool(name="blc", bufs=3) as blpool, \
             tc.tile_pool(name="acc", bufs=6, space="PSUM") as pacc, \
             tc.tile_pool(name="work", bufs=2, space="PSUM") as pwork:

            # ---- load constants ----
            def load(name, shape, dt=F32R, src=None):
                t = cpool.tile(shape, dt, tag=name)
                nc.sync.dma_start(t[:], src if src is not None else d[name].ap())
                return t
            ident = load("ident", [128, 128])
            sel16 = load("sel16", [32, 16, 128])
            sel64 = load("sel64", [64, 128])
            onesl = load("onesl", [128, 1])
            repl = load("repl", [1, 128], F32)
            smt = load("smt", [M, E])
            ghp = load("ghp", [E, NPC]); gtp = load("gtp", [E, NPC])
            hh = load("hh", [E, NPC]); htt = load("htt", [E, NPC])
            bh = load("bh", [128, 6], F32); bt = load("bt", [128, 6], F32)
            bbil = load("bbil", [128, 6], F32)
            bcls = load("bcls", [NC_, 1], F32); bbin = load("bbin", [1, 1], F32)
            eadd = load("eadd", [E, 1], F32)
            entl = load("entl", [M, H], F32)
            wcls = load("wcls", [128, 6, NC_],
                        src=d["wcls"].ap().rearrange("(c p) n -> p c n", p=128))
            wbin = load("wbin", [128, 6, 1],
                        src=d["wbin"].ap().rearrange("(c p) n -> p c n", p=128))

            # ---- pooling: ent_emb = ln(sum exp) ; asum = S @ attn ----
            exp_x = bpool.tile([M, H], F32R, tag="expx")
            nc.scalar.activation(exp_x[:], entl[:], Exp)
            lse = bpool.tile([E, H], F32R, tag="lse")
            for j in range(2):
                ps = pwork.tile([E, 512], F32, tag="w")
                nc.tensor.matmul(ps[:], smt[:], exp_x[:, 512 * j:512 * (j + 1)],
                                 start=True, stop=True)
                nc.scalar.activation(lse[:, 512 * j:512 * (j + 1)], ps[:], Ln,
                                     bias=eadd[:])

            # ---- ha/ta gathers + ht_un, streamed per attention head a ----
            htf = bpool.tile([128, 8, NPC], F32, tag="htf")      # f32 accumulator
            htun = bpool.tile([128, 8, NPC], F32R, tag="htun")   # final f32r
            for a in range(A):
                at = atpool.tile([M, 1024], F32R, tag="attn", name=f"at{a}")
                nc.sync.dma_start(at[:], d["attnT"].ap()[:, 1024 * a:1024 * (a + 1)])
                asb = prpool.tile([E, 1024], F32R, tag="asb", name=f"as{a}")
                for jj in range(2):
                    ps = pwork.tile([E, 512], F32, tag="w", name=f"as{a}_{jj}")
                    nc.tensor.matmul(ps[:], smt[:], at[:, 512 * jj:512 * (jj + 1)],
                                     start=True, stop=True)
                    nc.scalar.activation(asb[:, 512 * jj:512 * (jj + 1)], ps[:],
                                         Copy)
                for r in range(8):       # l-range
                    sl = asb[:, 128 * r:128 * r + 128]
                    pha = pacc.tile([128, NPC], F32, tag="acc", name=f"ha{a}_{r}")
                    nc.tensor.matmul(pha[:], sl, ghp[:], start=True, stop=True)
                    pta = pacc.tile([128, NPC], F32, tag="acc", name=f"ta{a}_{r}")
                    nc.tensor.matmul(pta[:], sl, gtp[:], start=True, stop=True)
                    hsb = prpool.tile([128, NPC], F32, tag="hsb", name=f"hs{a}_{r}")
                    nc.scalar.activation(hsb[:], pha[:], Copy)
                    if a == 0:
                        nc.vector.tensor_mul(htf[:, r, :], hsb[:], pta[:])
                    else:
                        p = prpool.tile([128, NPC], F32, tag="p", name=f"p{a}_{r}")
                        nc.vector.tensor_mul(p[:], hsb[:], pta[:])
                        dst = htun if a == A - 1 else htf
                        eng = nc.gpsimd if (a % 2 == 0 and a != A - 1) else nc.vector
                        eng.tensor_add(dst[:, r, :], htf[:, r, :], p[:])

            # ---- normalization scale: inv_s replicated over partitions ----
            sps = pwork.tile([1, NPC], F32, tag="w")
            for j in range(8):
                nc.tensor.matmul(sps[:], onesl[:], htun[:, j, :],
                                 start=(j == 0), stop=(j == 7))
            seps = supool.tile([1, NPC], F32, tag="s")
            nc.vector.tensor_scalar_add(seps[:], sps[:], 1e-5)
            invs = supool.tile([1, NPC], F32, tag="s")
            nc.vector.reciprocal(invs[:], seps[:])
            irep_ps = pwork.tile([128, NPC], F32, tag="w")
            nc.tensor.matmul(irep_ps[:], repl[:], invs[:], start=True, stop=True)
            irep = bpool.tile([128, NPC], F32, tag="irepsb")
            nc.scalar.activation(irep[:], irep_ps[:], Copy)

            # ---- rel (transposed) + scale ----
            relT = bpool.tile([128, 8, NPC], F32R, tag="relT")
            for og in range(4):
                sq = sqpool.tile([128, 8, 256], F32R, tag="seq")
                nc.sync.dma_start(
                    sq[:], d["seqd"].ap().rearrange("(c p) h -> p c h", p=128)
                    [:, :, 256 * og:256 * (og + 1)])
                for oo in range(2):
                    o = 2 * og + oo
                    ps = pwork.tile([128, NPC], F32, tag="w", name=f"rl{og}{oo}")
                    for j in range(8):
                        nc.tensor.matmul(ps[:], sq[:, j, 128 * oo:128 * (oo + 1)],
                                         htun[:, j, :], start=(j == 0), stop=(j == 7))
                    nc.vector.tensor_mul(relT[:, o, :], ps[:], irep[:])

            # ---- projections: hsT/tsT = tanh(W^T [ent;rel] + b) ----
            hsT = bpool.tile([128, 6, NPC], F32R, tag="hsT")
            tsT = bpool.tile([128, 6, NPC], F32R, tag="tsT")
            for wname, hot, dst, bias in (("whead", hh, hsT, bh),
                                          ("wtail", htt, tsT, bt)):
                accs = [pacc.tile([128, NPC], F32, tag="acc", name=f"pj{o}")
                        for o in range(6)]
                for j in range(16):
                    wc = spool.tile([128, EMB], F32R, tag="wstr")
                    nc.sync.dma_start(wc[:], d[wname].ap()[128 * j:128 * (j + 1), :])
                    if j < 8:
                        eps_ = pwork.tile([128, NPC], F32, tag="w",
                                          name=f"eg{wname}{j}")
                        nc.tensor.matmul(eps_[:], lse[:, 128 * j:128 * (j + 1)],
                                         hot[:], start=True, stop=True)
                        ent_sb = prpool.tile([128, NPC], F32R, tag="entg",
                                             name=f"eg2{wname}{j}")
                        nc.scalar.activation(ent_sb[:], eps_[:], Copy)
                        rhs = ent_sb[:]
                    else:
                        rhs = relT[:, j - 8, :]
                    for o in range(6):
                        nc.tensor.matmul(accs[o][:], wc[:, 128 * o:128 * (o + 1)],
                                         rhs, start=(j == 0), stop=(j == 15))
                for o in range(6):
                    nc.scalar.activation(dst[:, o, :], accs[o][:], Tanh,
                                         bias=bias[:, o:o + 1])

            # ---- bilinear GEMM: embT[o, n] = sum_c W[c-chunk]^T bl^T[c-chunk] ----
            eaccs = [pacc.tile([128, NPC], F32, tag="acc", name=f"ea{o}")
                     for o in range(6)]
            ts2c = None
            h2sb32 = None
            for c in range(NCHUNK):
                k = c // 32
                if c % 32 == 0:      # replicate ts k-block on both halves
                    t2a = pwork.tile([64, NPC], F32, tag="w", name=f"t2a{k}")
                    nc.tensor.matmul(t2a[:],
                                     ident[:, 64 * (k % 2):64 * (k % 2) + 64],
                                     tsT[:, k // 2, :], start=True, stop=True)
                    t2sb = blpool.tile([64, NPC], F32R, tag="t2sb", name=f"t2s{k}")
                    nc.vector.tensor_copy(t2sb[:], t2a[:])
                    tps = pwork.tile([128, NPC], F32, tag="w", name=f"tp{k}")
                    nc.tensor.matmul(tps[:], sel64[:], t2sb[:],
                                     start=True, stop=True)
                    ts2c = hrpool.tile([128, NPC], F32R, tag="ts2c", name=f"t2{k}")
                    nc.scalar.activation(ts2c[:], tps[:], Copy)
                wt = spool.tile([128, OUT], F32R, tag="wstr", name=f"wb{c}")
                nc.sync.dma_start(wt[:], d["wbil"].ap()[128 * c:128 * (c + 1), :])
                if c % 16 == 0:
                    q16 = c // 16
                    hp2 = pwork.tile([32, NPC], F32, tag="w", name=f"hp2_{q16}")
                    nc.tensor.matmul(hp2[:],
                                     ident[:, 32 * (q16 % 4):32 * (q16 % 4) + 32],
                                     hsT[:, q16 // 4, :], start=True, stop=True)
                    h2sb32 = blpool.tile([32, NPC], F32R, tag="h2sb",
                                         name=f"h2s{q16}")
                    nc.vector.tensor_copy(h2sb32[:], hp2[:])
                hp = pwork.tile([128, NPC], F32, tag="w", name=f"hp{c}")
                nc.tensor.matmul(hp[:], sel16[:, c % 16, :], h2sb32[:],
                                 start=True, stop=True)
                hr = hrpool.tile([128, NPC], F32R, tag="hr")
                nc.scalar.activation(hr[:], hp[:], Copy)
                bl = blpool.tile([128, NPC], F32R, tag="bl")
                nc.vector.tensor_mul(bl[:], ts2c[:], hr[:])
                for o in range(6):
                    nc.tensor.matmul(eaccs[o][:], wt[:, 128 * o:128 * (o + 1)],
                                     bl[:], start=(c == 0), stop=(c == NCHUNK - 1))

            # ---- outputs ----
            embT = bpool.tile([128, 6, NPC], F32R, tag="embT")
            for o in range(6):
                nc.vector.tensor_scalar_add(embT[:, o, :], eaccs[o][:],
                                            bbil[:, o:o + 1])
            nc.sync.dma_start(
                embd.ap().rearrange("(c p) n -> p c n", p=128),
                embT[:].bitcast(F32))
            pcls = pwork.tile([NC_, NPC], F32, tag="w")
            for j in range(6):
                nc.tensor.matmul(pcls[:], wcls[:, j, :], embT[:, j, :],
                                 start=(j == 0), stop=(j == 5))
            clsT = bpool.tile([NC_, NPC], F32, tag="clsT")
            nc.vector.tensor_scalar_add(clsT[:], pcls[:], bcls[:])
            nc.sync.dma_start(clsd.ap(), clsT[:])
            pbin = pwork.tile([1, NPC], F32, tag="w")
            for j in range(6):
                nc.tensor.matmul(pbin[:], wbin[:, j, :], embT[:, j, :],
                                 start=(j == 0), stop=(j == 5))
            binT = bpool.tile([1, NPC], F32, tag="binT")
            nc.vector.tensor_scalar_add(binT[:], pbin[:], bbin[:])
            nc.sync.dma_start(bind.ap(), binT[:])

    nc.compile()
    return nc


_NC_CACHE = None


def _get_program():
    global _NC_CACHE
    if _NC_CACHE is None:
        _NC_CACHE = build_program()
    return _NC_CACHE


def _one_hot_T(idx, n):
    """[n, len(idx)] one-hot: out[e, r] = (idx[r] == e)."""
    out = np.zeros((n, len(idx)), np.float32)
    out[idx, np.arange(len(idx))] = 1.0
    return out


def kernel(seq_lhs, ent_lhs, ent_to_seq_attn, entity_id_labels, hts,
           W_head, b_head, W_tail, b_tail, W_bil, b_bil,
           W_cls, b_cls, W_bin, b_bin):
    seq_lhs = np.asarray(seq_lhs, np.float32)
    ent_lhs = np.asarray(ent_lhs, np.float32)
    ent_to_seq_attn = np.asarray(ent_to_seq_attn, np.float32)
    entity_id_labels = np.asarray(entity_id_labels)
    hts = np.asarray(hts)

    nc = _get_program()

    shared = {
        "wbil": f32r_round(W_bil),
        "whead": f32r_round(W_head), "wtail": f32r_round(W_tail),
        "wcls": f32r_round(W_cls), "wbin": f32r_round(W_bin),
        "ident": np.eye(128, dtype=np.float32),
        "sel16": (np.arange(32)[:, None, None]
                  == 2 * np.arange(16)[None, :, None]
                  + (np.arange(128)[None, None, :] >= 64))
                 .astype(np.float32),
        "sel64": (np.arange(64)[:, None] == np.arange(128)[None, :] % 64)
                .astype(np.float32),
        "onesl": np.ones((128, 1), np.float32),
        "repl": np.ones((1, 128), np.float32),
        "bh": np.asarray(b_head, np.float32).reshape(6, 128).T.copy(),
        "bt": np.asarray(b_tail, np.float32).reshape(6, 128).T.copy(),
        "bbil": np.asarray(b_bil, np.float32).reshape(6, 128).T.copy(),
        "bcls": np.asarray(b_cls, np.float32).reshape(NC_, 1),
        "bbin": np.asarray(b_bin, np.float32).reshape(1, 1),
    }
    in_maps = []
    for c in range(8):
        dcc, h = divmod(c, 2)
        labels = entity_id_labels[dcc]
        cnt = np.bincount(labels, minlength=E).astype(np.float32)
        inv_cnt = 1.0 / np.maximum(cnt, 1.0)
        p0 = NPC * h
        hd = hts[dcc, p0:p0 + NPC, 0]
        td = hts[dcc, p0:p0 + NPC, 1]
        m = dict(shared)
        m["seqd"] = f32r_round(seq_lhs[dcc])
        m["attnT"] = f32r_round(
            ent_to_seq_attn[dcc].transpose(1, 0, 2).reshape(M, A * L))
        m["entl"] = ent_lhs[dcc]
        m["smt"] = _one_hot_T(labels, E).T.copy()     # [M, E]
        m["eadd"] = (cnt == 0).astype(np.float32).reshape(E, 1)
        m["ghp"] = f32r_round(_one_hot_T(hd, E) * (inv_cnt / A)[:, None])
        m["gtp"] = f32r_round(_one_hot_T(td, E) * inv_cnt[:, None])
        m["hh"] = _one_hot_T(hd, E)
        m["htt"] = _one_hot_T(td, E)
        in_maps.append(m)

    res = run_bass_kernel_spmd(nc, in_maps, core_ids=list(range(8)))

    embeds = np.empty((B * R, OUT), np.float32)
    cls_l = np.empty((B * R, NC_), np.float32)
    bin_l = np.empty((B * R, 1), np.float32)
    for c in range(8):
        dcc, h = divmod(c, 2)
        r0 = R * dcc + NPC * h
        embeds[r0:r0 + NPC] = res.results[c]["embT"].T
        cls_l[r0:r0 + NPC] = res.results[c]["clsT"].T
        bin_l[r0:r0 + NPC] = res.results[c]["binT"].T
    return embeds, cls_l, bin_l
